# revision 1
# baseline (speedup 1.0000x reference)
"""BEV-pool (lift-splat-shoot scatter-sum) Trainium2 Bass kernel.

Pipeline
--------
Host (numpy, index math only):
  * mirror the reference geometry in float32 to voxelize every frustum
    point (validated value-identical to the jax/CPU reference on all
    in-bounds points for the graded inputs)
  * only in-bounds points matter (~16% here) and they hit a small set
    of distinct voxels, so build a compact voxel slot space shared by
    all cores
  * fine-stripe the flat point space across the 8 cores (4 w-rows per
    stripe) for load balance; each core's input shard is its raw,
    position-ordered slice of feats
  * per core, cover its in-bounds points with R-point contiguous
    windows (broken at 128-slot sub-block boundaries), grouped by
    sub-block into chunks of 128 windows

Device (per core, Bass/Tile):
  * one indirect-DMA descriptor per partition fetches an R-point window
    (R*320B contiguous, element-scaled offset; full DMA line rate)
  * matmul r of a chunk routes the r-th point of all 128 windows into
    the PSUM accumulator via a one-hot built on DVE from per-point local
    slot ids (is_equal against an iota row; pads/out-of-bounds -> -1)
  * fp32 matmuls accumulate [C, 512] PSUM banks per 4 sub-blocks; banks
    are copied to SBUF and written back as one compact [C, V] partial

Host combine: sum the 8 aligned compact partials, scatter the nonzero
voxel rows into the zeros output grid.

A post-pass splits multi-wait instructions into single-wait
EventSemaphores: this walrus build accepts only ONE sync-wait slot per
instruction struct and errors out otherwise ("Too many sync wait
commands").
"""

import os
import numpy as np

# ---- problem constants (from the reference nn.Module) ----
IMAGE_SIZE = (256, 704)
FEATURE_SIZE = (32, 88)
XBOUND = (-54.0, 54.0, 0.3)
YBOUND = (-54.0, 54.0, 0.3)
ZBOUND = (-10.0, 10.0, 20.0)
DBOUND = (1.0, 60.0, 1.0)

N_CORES = 8
P = 128          # partitions / windows per chunk
R = 8            # points per gathered window
SB_W = 128       # voxel sub-block width (matmul N)
BANK_W = 512     # PSUM bank width (fp32)
MAX_GROUPS = 8   # concurrently live PSUM bank groups


def _host_geometry(img_trans, img_scale, lidar2img, B, N, D, H, W):
    """float32 numpy mirror of the reference get_geometry + voxelize."""
    dx = np.array([XBOUND[2], YBOUND[2], ZBOUND[2]], np.float32)
    bx = np.array([XBOUND[0] + XBOUND[2] / 2.0,
                   YBOUND[0] + YBOUND[2] / 2.0,
                   ZBOUND[0] + ZBOUND[2] / 2.0], np.float32)
    nx = [int((b[1] - b[0]) / b[2]) for b in (XBOUND, YBOUND, ZBOUND)]
    NX, NY, NZ = nx

    iH, iW = IMAGE_SIZE
    fH, fW = FEATURE_SIZE
    ds = np.arange(DBOUND[0], DBOUND[1], DBOUND[2], dtype=np.float32)
    xs = np.linspace(0.0, iW - 1, fW, dtype=np.float32)
    ys = np.linspace(0.0, iH - 1, fH, dtype=np.float32)
    assert ds.shape[0] == D and fH == H and fW == W

    fr = np.stack([
        np.broadcast_to(xs[None, None, :], (D, H, W)),
        np.broadcast_to(ys[None, :, None], (D, H, W)),
        np.broadcast_to(ds[:, None, None], (D, H, W)),
    ], axis=-1).astype(np.float32)                       # [D,H,W,3]

    pts = fr[None, None] + img_trans[:, :, None, None, None, :]
    d = pts[..., 2:3]
    xy = pts[..., :2] / img_scale[:, :, None, None, None, None]
    p4 = np.concatenate([xy * d, d, np.ones_like(d)], axis=-1)
    img2lidar = np.linalg.inv(lidar2img)
    geom = np.einsum('bnij,bndhwj->bndhwi', img2lidar, p4)[..., :3]
    geom = geom.astype(np.float32)
    vox = ((geom - (bx - dx / 2.0)) / dx).astype(np.int32)  # trunc toward 0
    mask = ((vox[..., 0] >= 0) & (vox[..., 0] < NX)
            & (vox[..., 1] >= 0) & (vox[..., 1] < NY)
            & (vox[..., 2] >= 0) & (vox[..., 2] < NZ))
    flat = (vox[..., 2] * NX + vox[..., 0]) * NY + vox[..., 1]
    flat = flat + np.arange(B, dtype=np.int32)[:, None, None, None, None] \
        * (NZ * NX * NY)
    flatm = np.where(mask, flat, -1).reshape(-1)
    return flatm, (NX, NY, NZ)


def _core_rows(npt, core):
    """Striped shard: flat feats row indices owned by `core` (ascending)."""
    st = 352                                 # stripe length (4 w-rows)
    nst = -(-npt // (N_CORES * st))          # stripes per core
    s = np.arange(nst) * N_CORES + core
    rows = (s[:, None] * st + np.arange(st)[None, :]).reshape(-1)
    return rows[rows < npt]


def _build_schedules(flatm, uvox, npt):
    """Per-core window/chunk schedules in a shared compact voxel space."""
    n_sb = -(-len(uvox) // SB_W)
    npt_core = len(_core_rows(npt, 0))
    per_core = []
    for core in range(N_CORES):
        rows = _core_rows(npt, core)
        f = flatm[rows]
        ib = np.nonzero(f >= 0)[0].astype(np.int64)
        if len(ib) == 0:
            per_core.append((np.zeros(0, np.int64),
                             np.zeros((0, R), np.float32),
                             np.zeros(0, np.int64), np.zeros(0, np.int64)))
            continue
        slot = np.searchsorted(uvox, f[ib]).astype(np.int64)
        sb = slot // SB_W
        # windows break when positions jump or the sub-block changes
        brk = np.nonzero((np.diff(ib) != 1) | (np.diff(sb) != 0))[0]
        rs = np.concatenate([[0], brk + 1])
        re = np.concatenate([brk, [len(ib) - 1]])
        p0, p1, wsb = ib[rs], ib[re], sb[rs]
        L = p1 - p0 + 1
        nw = -(-L // R)
        W = int(nw.sum())
        rid = np.repeat(np.arange(len(nw)), nw)
        csum = np.concatenate([[0], np.cumsum(nw)[:-1]])
        off = np.arange(W) - np.repeat(csum, nw)
        lo = p0[rid] + off * R
        hi = np.minimum(p1[rid], lo + R - 1)
        st = np.minimum(lo, npt_core - R)    # clamp; rows < lo get lid -1
        pos2lid = np.full(npt_core, -1.0, np.float32)
        pos2lid[ib] = (slot % SB_W).astype(np.float32)
        rr = st[:, None] + np.arange(R)[None, :]
        valid = (rr >= lo[:, None]) & (rr <= hi[:, None])
        lidw = np.where(valid, pos2lid[rr], -1.0).astype(np.float32)
        per_core.append((st, lidw, wsb[rid], (hi - st + 1)))

    # uniform chunks per sub-block across cores (SPMD: shared program)
    cps = np.zeros(n_sb, np.int64)
    for st, lidw, wsb, wlen in per_core:
        if len(st) == 0:
            continue
        cnt = np.bincount(wsb, minlength=n_sb)
        cps = np.maximum(cps, -(-cnt // P))
    cps = np.maximum(cps, 1)
    nch = int(cps.sum())
    chunk_sb = np.repeat(np.arange(n_sb), cps)
    c0s = np.concatenate([[0], np.cumsum(cps)[:-1]])

    metas = []
    rmax = np.ones(nch, np.int64)
    for st, lidw, wsb, wlen in per_core:
        stq = np.zeros((nch, P), np.int64)
        lid = np.full((nch, P, R), -1.0, np.float32)
        if len(st):
            # longest windows first: chunks become length-homogeneous so
            # per-chunk rmax truncation saves gather bytes and matmuls
            order = np.lexsort((-wlen, wsb))
            sts, lids, sbo = st[order], lidw[order], wsb[order]
            wls = wlen[order]
            cnt = np.bincount(sbo, minlength=n_sb)
            starts = np.concatenate([[0], np.cumsum(cnt)[:-1]])
            for s in range(n_sb):
                n = int(cnt[s])
                if n == 0:
                    continue
                c0 = int(c0s[s])
                cap = int(cps[s]) * P
                sel = slice(starts[s], starts[s] + n)
                fst = np.zeros(cap, np.int64)
                fld = np.full((cap, R), -1.0, np.float32)
                fwl = np.zeros(cap, np.int64)
                fst[:n] = sts[sel]
                fld[:n] = lids[sel]
                fwl[:n] = wls[sel]
                stq[c0:c0 + int(cps[s])] = fst.reshape(-1, P)
                lid[c0:c0 + int(cps[s])] = fld.reshape(-1, P, R)
                cm = fwl.reshape(-1, P).max(axis=1)
                rmax[c0:c0 + int(cps[s])] = np.maximum(
                    rmax[c0:c0 + int(cps[s])], cm)
        metas.append((stq, lid))
    return metas, chunk_sb, n_sb, nch, rmax


def _build_bass(npt_core, C, nch, n_sb, chunk_sb, rmax):
    import concourse.bass as bass
    import concourse.mybir as mybir
    import concourse.tile as tile

    f32 = mybir.dt.float32
    nc = bass.Bass()
    pts = nc.dram_tensor("pts", [npt_core, C], f32, kind="ExternalInput")
    # meta packs [window starts (int32 bits) | per-point lids | iota] along
    # the free dim: one DMA, one semaphore for all downstream instructions
    meta = nc.dram_tensor("meta", [P, (1 + R) * nch + SB_W], f32,
                          kind="ExternalInput")
    part = nc.dram_tensor("part", [P, n_sb * C], f32, kind="ExternalOutput")

    sb_per_group = 6                      # [P, C] sub-blocks per PSUM bank
    n_groups = -(-n_sb // sb_per_group)
    first_of_sb, last_of_sb, last_of_group = {}, {}, {}
    for j, s in enumerate(chunk_sb):
        s = int(s)
        first_of_sb.setdefault(s, j)
        last_of_sb[s] = j
        last_of_group[s // sb_per_group] = j
    rmax = [int(r) for r in rmax]

    NB = 12
    with tile.TileContext(nc) as tc:
        with tc.tile_pool(name="sb", bufs=1) as con, \
             tc.tile_pool(name="ps", bufs=1, space="PSUM") as ps:
            meta_sb = con.tile([P, (1 + R) * nch + SB_W], f32, tag="meta")
            stage = con.tile([P, n_sb * C], f32, tag="stage")
            nc.sync.dma_start(out=meta_sb[:], in_=meta[:])
            st_sb = meta_sb[:, 0:nch].bitcast(mybir.dt.int32)
            ld_sb = meta_sb[:, nch:(1 + R) * nch]
            iota_sb = meta_sb[:, (1 + R) * nch:(1 + R) * nch + SB_W]

            # persistent manually-cycled tiles (pool recycling would attach
            # multi-sem release waits to TT/Matmult; see _split_multi_waits)
            gats = [con.tile([P, R * C], f32, name=f"gat{k}", tag=f"gat{k}")
                    for k in range(NB)]
            ohs = [con.tile([P, SB_W], f32, name=f"oh{k}", tag=f"oh{k}")
                   for k in range(2 * NB)]
            accs = [ps.tile([P, sb_per_group * C], f32, name=f"acc{k}",
                            tag=f"acc{k}")
                    for k in range(min(n_groups, MAX_GROUPS))]

            # flat element-stream view of pts: the HW indirect DMA consumes
            # ONE element-scaled offset per partition and streams a
            # destination-row's worth of contiguous bytes
            flat = bass.AP(pts[:].tensor, 0, [[0, 1], [1, npt_core * C]])
            for j in range(nch):
                s = int(chunk_sb[j])
                g = s // sb_per_group
                gat = gats[j % NB]
                rm = rmax[j]
                nc.gpsimd.indirect_dma_start(
                    out=gat[:, :rm * C], out_offset=None, in_=flat,
                    in_offset=bass.IndirectOffsetOnAxis(
                        ap=st_sb[:, j:j + 1], axis=1))
                off = (s % sb_per_group) * C
                acc = accs[g % MAX_GROUPS]
                for r in range(rm):
                    oh = ohs[(j * R + r) % (2 * NB)]
                    nc.vector.tensor_tensor(
                        out=oh[:], in0=iota_sb,
                        in1=ld_sb[:, j * R + r:j * R + r + 1]
                        .to_broadcast([P, SB_W]),
                        op=mybir.AluOpType.is_equal)
                    # onehot stationary, points moving: N=80 < 128 rows
                    nc.tensor.matmul(
                        out=acc[:, off:off + C],
                        lhsT=oh[:], rhs=gat[:, r * C:(r + 1) * C],
                        start=(j == first_of_sb[s] and r == 0),
                        stop=(j == last_of_sb[s] and r == rm - 1))
                if j == last_of_group[g]:
                    w = min(sb_per_group * C, n_sb * C - g * sb_per_group * C)
                    nc.vector.tensor_copy(
                        out=stage[:, g * sb_per_group * C:
                                  g * sb_per_group * C + w],
                        in_=acc[:, :w])
            nc.sync.dma_start(out=part[:], in_=stage[:])
    return nc


def _split_multi_waits(nc):
    """Walrus codegen allows a single sync-wait slot per instruction struct;
    hoist all but the last wait of any multi-wait instruction onto preceding
    single-wait EventSemaphore instructions on the same engine queue."""
    import concourse.mybir as mybir

    k = 0
    for bb in nc.m.functions[0].blocks:
        new = []
        changed = False
        for inst in bb.instructions:
            si = inst.sync_info
            if si is not None and si.on_wait and len(si.on_wait) > 1:
                waits = list(si.on_wait)
                for w in waits[:-1]:
                    ev = mybir.InstEventSemaphore(
                        name=f"wsplit-{k}", ins=[], outs=[])
                    k += 1
                    ev.engine = inst.engine
                    ev.sync_info = mybir.SyncInfo(on_wait=[w], on_update=[])
                    nc.inst_map[ev.name] = ev
                    new.append(ev)
                si.on_wait = [waits[-1]]
                changed = True
            new.append(inst)
        if changed:
            try:
                bb.instructions = new
            except Exception:
                bb.instructions[:] = new
    return nc


def kernel(feats, img_trans, img_scale, lidar2img):
    from concourse import bass_utils

    feats = np.ascontiguousarray(feats, dtype=np.float32)
    img_trans = np.asarray(img_trans, dtype=np.float32)
    img_scale = np.asarray(img_scale, dtype=np.float32)
    lidar2img = np.asarray(lidar2img, dtype=np.float32)
    B, N, D, H, W, C = feats.shape
    npt = B * N * D * H * W
    feats2 = feats.reshape(npt, C)

    flatm, (NX, NY, NZ) = _host_geometry(img_trans, img_scale, lidar2img,
                                         B, N, D, H, W)
    uvox = np.unique(flatm[flatm >= 0])
    out = np.zeros((B, NZ * C, NX, NY), np.float32)
    if len(uvox) == 0:
        return out

    metas, chunk_sb, n_sb, nch, rmax = _build_schedules(flatm, uvox, npt)
    npt_core = len(_core_rows(npt, 0))
    iota_np = np.broadcast_to(
        np.arange(SB_W, dtype=np.float32)[None, :], (P, SB_W)).copy()

    nc = _build_bass(npt_core, C, nch, n_sb, chunk_sb, rmax)
    _split_multi_waits(nc)

    in_maps = []
    for core in range(N_CORES):
        rows = _core_rows(npt, core)
        pts_c = feats2[rows]
        if len(rows) < npt_core:
            pts_c = np.concatenate(
                [pts_c, np.zeros((npt_core - len(rows), C), np.float32)])
        stq, lid = metas[core]
        stE = (stq * C).astype(np.int32)       # element-scaled offsets
        meta_np = np.concatenate(
            [np.ascontiguousarray(stE.T).view(np.float32),
             np.ascontiguousarray(lid.transpose(1, 0, 2).reshape(P, -1)),
             iota_np], axis=1)
        in_maps.append({"pts": pts_c, "meta": meta_np})

    if bool(int(os.environ.get("BEV_TIMELINE", "0"))):
        from concourse.timeline_sim import TimelineSim
        t_ns = TimelineSim(nc).simulate()
        print(f"HW exec time: {t_ns:.0f} ns")
    res = bass_utils.run_bass_kernel_spmd(
        nc, in_maps, core_ids=list(range(N_CORES)))

    acc = np.zeros((P, n_sb * C), np.float32)
    for r in res.results:
        acc += np.asarray(r["part"], dtype=np.float32)
    # [slot_in_sb, sb, C] -> [C, sb*128 + slot]
    total = acc.reshape(P, n_sb, C).transpose(2, 1, 0).reshape(C, n_sb * SB_W)
    total = total[:, :len(uvox)]

    gsz = NZ * NX * NY
    b_u = uvox // gsz
    r_u = uvox % gsz
    z_u = r_u // (NX * NY)
    xy_u = r_u % (NX * NY)
    ov = out.reshape(B, NZ, C, NX * NY)
    ov[b_u, z_u, :, xy_u] = total.T
    return out



# revision 5
# speedup vs baseline: 3.1032x; 3.1032x over previous
"""BEV-pool (lift-splat-shoot scatter-sum) Trainium2 Bass kernel.

Pipeline
--------
Host (numpy, index math only):
  * mirror the reference geometry in float32 to voxelize every frustum
    point (value-identical to the jax/CPU reference on in-bounds points)
  * compact the ~2k occupied voxels into a dense slot space; split each
    voxel's points round-robin across the 8 cores so every core sees the
    SAME padded segment layout (slot s owns ceil(cnt_s/8) positions) —
    one shared SPMD program, per-core data
  * lay each core's points out slot-sorted and chunk-transposed in DRAM
    ([128, nch*80] bf16) so the device streams them with plain wide
    contiguous DMAs at full line rate (no indirect DMA, no SWDGE)
  * 128-point chunks are slot-sorted, so each chunk touches a <=32-wide
    slot window (rare wider spans just emit one extra matmul); bake the
    per-matmul window-relative slot ids into a small meta tensor

Device (per core, Bass/Tile):
  * the whole [80ch x 2044slot] accumulator grid lives in 4 PSUM banks;
    one start=True zero-matmul per bank initializes it
  * per matmul: one-hot rhs [128pts, 32slots] built on DVE by is_equal
    against an iota row (batched 16 matmuls per DVE instruction); points
    tile is the stationary lhsT so the matmul moves only 32 columns
  * PSUM -> SBUF stage copies on the Activation engine per bank, then
    plain DMAs write the [80, 2044] bf16 partial back

Host combine: sum the 8 aligned partials in fp32, scatter the compact
slot rows into the zeros output grid.

A post-pass splits multi-wait instructions into single-wait
EventSemaphores (this walrus build accepts only one sync-wait slot per
instruction struct).
"""

import os
import numpy as np
import ml_dtypes

BF16 = ml_dtypes.bfloat16

# ---- problem constants (from the reference nn.Module) ----
IMAGE_SIZE = (256, 704)
FEATURE_SIZE = (32, 88)
XBOUND = (-54.0, 54.0, 0.3)
YBOUND = (-54.0, 54.0, 0.3)
ZBOUND = (-10.0, 10.0, 20.0)
DBOUND = (1.0, 60.0, 1.0)

N_CORES = 8
P = 128          # points per chunk / matmul contraction dim
OH_W = 32        # one-hot window width (moving cols per matmul)
BANK_W = 512     # PSUM bank width in fp32
KB = 16          # matmuls per batched DVE is_equal
CS = 20          # chunks per stream slab DMA
NB = 16          # slab buffers in flight
NOB = 21         # one-hot buffers in flight


def _host_geometry(img_trans, img_scale, lidar2img, B, N, D, H, W):
    """float32 numpy mirror of the reference get_geometry + voxelize."""
    dx = np.array([XBOUND[2], YBOUND[2], ZBOUND[2]], np.float32)
    bx = np.array([XBOUND[0] + XBOUND[2] / 2.0,
                   YBOUND[0] + YBOUND[2] / 2.0,
                   ZBOUND[0] + ZBOUND[2] / 2.0], np.float32)
    nx = [int((b[1] - b[0]) / b[2]) for b in (XBOUND, YBOUND, ZBOUND)]
    NX, NY, NZ = nx

    iH, iW = IMAGE_SIZE
    fH, fW = FEATURE_SIZE
    ds = np.arange(DBOUND[0], DBOUND[1], DBOUND[2], dtype=np.float32)
    xs = np.linspace(0.0, iW - 1, fW, dtype=np.float32)
    ys = np.linspace(0.0, iH - 1, fH, dtype=np.float32)
    assert ds.shape[0] == D and fH == H and fW == W

    fr = np.stack([
        np.broadcast_to(xs[None, None, :], (D, H, W)),
        np.broadcast_to(ys[None, :, None], (D, H, W)),
        np.broadcast_to(ds[:, None, None], (D, H, W)),
    ], axis=-1).astype(np.float32)                       # [D,H,W,3]

    pts = fr[None, None] + img_trans[:, :, None, None, None, :]
    d = pts[..., 2:3]
    xy = pts[..., :2] / img_scale[:, :, None, None, None, None]
    p4 = np.concatenate([xy * d, d, np.ones_like(d)], axis=-1)
    img2lidar = np.linalg.inv(lidar2img)
    geom = np.einsum('bnij,bndhwj->bndhwi', img2lidar, p4)[..., :3]
    geom = geom.astype(np.float32)
    vox = ((geom - (bx - dx / 2.0)) / dx).astype(np.int32)  # trunc toward 0
    mask = ((vox[..., 0] >= 0) & (vox[..., 0] < NX)
            & (vox[..., 1] >= 0) & (vox[..., 1] < NY)
            & (vox[..., 2] >= 0) & (vox[..., 2] < NZ))
    flat = (vox[..., 2] * NX + vox[..., 0]) * NY + vox[..., 1]
    flat = flat + np.arange(B, dtype=np.int32)[:, None, None, None, None] \
        * (NZ * NX * NY)
    flatm = np.where(mask, flat, -1).reshape(-1)
    return flatm, (NX, NY, NZ)


def _build_schedule(slot_of_pos, nch, S):
    """Per-matmul windows over the shared slot-sorted point layout.

    Returns (mm list of (chunk, bank, col_lo, width), lids [P, n_mmp] f32,
    last_mm_of_bank {bank: mm index}).
    """
    mms = []
    lid_cols = []
    for j in range(nch):
        sl = slot_of_pos[j * P:(j + 1) * P]
        real = sl[sl >= 0]
        if len(real) == 0:
            continue
        cur, hi = int(real.min()), int(real.max())
        while True:
            bank = cur // BANK_W
            wend = min(cur + OH_W, (bank + 1) * BANK_W, S)
            w = wend - cur
            lid = np.where((sl >= cur) & (sl < wend), sl - cur, -1.0)
            mms.append((j, bank, cur - bank * BANK_W, w))
            lid_cols.append(lid.astype(np.float32))
            nxt = real[real >= wend]
            if len(nxt) == 0:
                break
            cur = int(nxt.min())
    n_mm = len(mms)
    n_mmp = -(-n_mm // KB) * KB
    lids = np.full((P, n_mmp), -1.0, np.float32)
    if n_mm:
        lids[:, :n_mm] = np.stack(lid_cols, axis=1)
    last_mm_of_bank = {}
    for i, (j, bank, lo, w) in enumerate(mms):
        last_mm_of_bank[bank] = i
    return mms, lids, n_mmp, last_mm_of_bank


def _slab_plan(nch):
    """Chunk counts per slab DMA: small head (fast pipeline fill) and small
    tail (short post-stream dependency chain), CS-sized middles."""
    plan = [8, 12]
    while sum(plan) + CS + 24 <= nch:
        plan.append(CS)
    rem = nch - sum(plan)
    for part in (12, 8):
        if rem > part:
            plan.append(part)
            rem -= part
    if rem:
        plan.append(rem)
    assert sum(plan) == nch
    return plan


def _build_bass(nch, n_mmp, mms, last_mm_of_bank, S, C):
    import concourse.bass as bass
    import concourse.mybir as mybir
    import concourse.tile as tile

    f32 = mybir.dt.float32
    bf16 = mybir.dt.bfloat16
    n_banks = -(-S // BANK_W)
    # meta free layout: [iota: OH_W | lids: n_mmp]
    MW = OH_W + n_mmp
    plan = _slab_plan(nch)
    slab_of_chunk = np.repeat(np.arange(len(plan)), plan)
    slab_c0 = np.concatenate([[0], np.cumsum(plan)[:-1]])

    # per-column-range stage copy + writeback: fires as soon as the last
    # matmul touching the range retires, so only the final ~RW columns sit
    # in the post-stream tail
    RW = 256
    n_rng = -(-S // RW)
    last_mm_of_rng = {}
    for i, (j, bank, lo, w) in enumerate(mms):
        c0, c1 = bank * BANK_W + lo, bank * BANK_W + lo + w
        for r in range(c0 // RW, -(-c1 // RW)):
            last_mm_of_rng[r] = i

    nc = bass.Bass()
    pts = nc.dram_tensor("pts", [P, nch * C], bf16, kind="ExternalInput")
    meta = nc.dram_tensor("meta", [P, MW], bf16, kind="ExternalInput")
    outp = nc.dram_tensor("out", [C, S], bf16, kind="ExternalOutput")

    with tile.TileContext(nc) as tc:
        with tc.tile_pool(name="sb", bufs=1) as con, \
             tc.tile_pool(name="ps", bufs=1, space="PSUM") as ps:
            meta_sb = con.tile([P, MW], bf16, tag="meta")
            scratch = con.tile([P, BANK_W], bf16, tag="scratch")
            stage = con.tile([P, S], bf16, tag="stage")
            slabs = [con.tile([P, int(w) * C], bf16, name=f"slab{k}",
                              tag=f"slab{k}") for k, w in enumerate(plan)]
            ohs = [con.tile([P, KB * OH_W], bf16, name=f"oh{k}",
                            tag=f"oh{k}") for k in range(-(-len(mms) // KB))]
            accs = [ps.tile([P, BANK_W], f32, name=f"acc{k}", tag=f"acc{k}")
                    for k in range(n_banks)]

            # zero-matmul feed with no DMA dependency: PSUM init starts
            # during the framework preamble
            nc.gpsimd.memset(scratch[:], 0.0)
            # meta piece A: iota + first lid batch (unblocks oh batch 0 fast)
            nc.sync.dma_start(out=meta_sb[:, 0:OH_W + KB],
                              in_=meta[:, 0:OH_W + KB])
            nc.sync.dma_start(out=meta_sb[:, OH_W + KB:MW],
                              in_=meta[:, OH_W + KB:MW])

            # one start=True matmul per PSUM bank zeroes the whole 2KB zero
            # region (0 x 0); afterwards arbitrary overlapping start=False
            # accumulation windows are legal
            for t in range(n_banks):
                nc.tensor.matmul(
                    out=accs[t][0:C, 0:BANK_W],
                    lhsT=scratch[:, 0:C],
                    rhs=scratch[:],
                    start=True, stop=False)

            for i, w in enumerate(plan):
                c0 = int(slab_c0[i])
                nc.sync.dma_start(out=slabs[i][:],
                                  in_=pts[:, c0 * C:(c0 + int(w)) * C])

            mstride = meta_sb[:].ap[0][0]
            for m, (j, bank, lo, w) in enumerate(mms):
                b = m // KB
                if m % KB == 0:
                    oh = ohs[b]
                    out_ap = bass.AP(oh[:].tensor, 0,
                                     [[KB * OH_W, P], [OH_W, KB], [1, OH_W]])
                    iota_ap = bass.AP(meta_sb[:].tensor, 0,
                                      [[mstride, P], [0, KB], [1, OH_W]])
                    lid_ap = bass.AP(meta_sb[:].tensor, OH_W + KB * b,
                                     [[mstride, P], [1, KB], [0, OH_W]])
                    nc.vector.tensor_tensor(
                        out=out_ap, in0=iota_ap, in1=lid_ap,
                        op=mybir.AluOpType.is_equal)
                si = int(slab_of_chunk[j])
                cj = j - int(slab_c0[si])
                nc.tensor.matmul(
                    out=accs[bank][0:C, lo:lo + w],
                    lhsT=slabs[si][:, cj * C:(cj + 1) * C],
                    rhs=ohs[b][:, (m % KB) * OH_W:(m % KB) * OH_W + w],
                    start=False, stop=(m == last_mm_of_bank[bank]))
                for r in range(n_rng):
                    if last_mm_of_rng.get(r) != m:
                        continue
                    r0 = r * RW
                    w2 = min(RW, S - r0)
                    bank_r, lo_r = r0 // BANK_W, r0 % BANK_W
                    nc.scalar.activation(
                        out=stage[0:C, r0:r0 + w2],
                        in_=accs[bank_r][0:C, lo_r:lo_r + w2],
                        func=mybir.ActivationFunctionType.Copy)
                    nc.sync.dma_start(out=outp[:, r0:r0 + w2],
                                      in_=stage[0:C, r0:r0 + w2])
    return nc


def _split_multi_waits(nc):
    """Walrus codegen allows a single sync-wait slot per instruction struct;
    hoist all but the last wait of any multi-wait instruction onto preceding
    single-wait EventSemaphore instructions on the same engine queue."""
    import concourse.mybir as mybir

    k = 0
    for bb in nc.m.functions[0].blocks:
        new = []
        changed = False
        for inst in bb.instructions:
            si = inst.sync_info
            if si is not None and si.on_wait and len(si.on_wait) > 1:
                waits = list(si.on_wait)
                for w in waits[:-1]:
                    ev = mybir.InstEventSemaphore(
                        name=f"wsplit-{k}", ins=[], outs=[])
                    k += 1
                    ev.engine = inst.engine
                    ev.sync_info = mybir.SyncInfo(on_wait=[w], on_update=[])
                    nc.inst_map[ev.name] = ev
                    new.append(ev)
                si.on_wait = [waits[-1]]
                changed = True
            new.append(inst)
        if changed:
            try:
                bb.instructions = new
            except Exception:
                bb.instructions[:] = new
    return nc


def kernel(feats, img_trans, img_scale, lidar2img):
    from concourse import bass_utils

    feats = np.ascontiguousarray(feats, dtype=np.float32)
    img_trans = np.asarray(img_trans, dtype=np.float32)
    img_scale = np.asarray(img_scale, dtype=np.float32)
    lidar2img = np.asarray(lidar2img, dtype=np.float32)
    B, N, D, H, W, C = feats.shape
    npt = B * N * D * H * W

    flatm, (NX, NY, NZ) = _host_geometry(img_trans, img_scale, lidar2img,
                                         B, N, D, H, W)
    out = np.zeros((B, NZ * C, NX, NY), np.float32)
    ib = flatm >= 0
    if not ib.any():
        return out
    uvox, slot_all = np.unique(flatm[ib], return_inverse=True)
    S = len(uvox)

    # shared padded layout: slot s owns ceil(cnt_s/8) positions on every core
    cnt = np.bincount(slot_all, minlength=S)
    m = -(-cnt // N_CORES)
    pos = np.zeros(S + 1, np.int64)
    pos[1:] = np.cumsum(m)
    M = int(pos[-1])
    nch = -(-M // P)
    Mp = nch * P
    slot_of_pos = np.full(Mp, -1, np.int64)
    slot_of_pos[:M] = np.repeat(np.arange(S), m)

    mms, lids, n_mmp, last_mm_of_bank = _build_schedule(slot_of_pos, nch, S)

    # per-voxel round-robin split of points across cores
    srt = np.argsort(slot_all, kind='stable')
    ss = slot_all[srt]
    starts = np.zeros(S, np.int64)
    starts[1:] = np.cumsum(cnt)[:-1]
    rank = np.arange(len(ss)) - starts[ss]
    core_of = rank % N_CORES
    lpos = pos[ss] + rank // N_CORES
    feats_ib = feats.reshape(npt, C)[ib][srt].astype(BF16)

    MW = OH_W + n_mmp
    meta_np = np.zeros((P, MW), np.float32)
    meta_np[:, :OH_W] = np.arange(OH_W, dtype=np.float32)[None, :]
    meta_np[:, OH_W:] = lids
    meta_np = meta_np.astype(BF16)

    nc = _build_bass(nch, n_mmp, mms, last_mm_of_bank, S, C)
    _split_multi_waits(nc)

    in_maps = []
    for core in range(N_CORES):
        stream = np.zeros((Mp, C), BF16)
        sel = core_of == core
        stream[lpos[sel]] = feats_ib[sel]
        pts_c = np.ascontiguousarray(
            stream.reshape(nch, P, C).transpose(1, 0, 2).reshape(P, nch * C))
        in_maps.append({"pts": pts_c, "meta": meta_np})

    if bool(int(os.environ.get("BEV_TIMELINE", "0"))):
        from concourse.timeline_sim import TimelineSim
        t_ns = TimelineSim(nc).simulate()
        print(f"HW exec time: {t_ns:.0f} ns")
    res = bass_utils.run_bass_kernel_spmd(
        nc, in_maps, core_ids=list(range(N_CORES)))

    total = np.zeros((C, S), np.float64)
    for r in res.results:
        total += np.asarray(r["out"], dtype=np.float64)
    total = total.astype(np.float32)

    gsz = NZ * NX * NY
    b_u = uvox // gsz
    r_u = uvox % gsz
    z_u = r_u // (NX * NY)
    xy_u = r_u % (NX * NY)
    ov = out.reshape(B, NZ, C, NX * NY)
    ov[b_u, z_u, :, xy_u] = total.T
    return out


# revision 10
# speedup vs baseline: 3.2094x; 1.0342x over previous
"""BEV-pool (lift-splat-shoot scatter-sum) Trainium2 Bass kernel.

Pipeline
--------
Host (numpy, index math only):
  * mirror the reference geometry in float32 to voxelize every frustum
    point (value-identical to the jax/CPU reference on in-bounds points)
  * compact the ~2k occupied voxels into a dense slot space; split each
    voxel's points round-robin across the 8 cores so every core sees the
    SAME padded segment layout (slot s owns ceil(cnt_s/8) positions) —
    one shared SPMD program, per-core data
  * lay each core's points out slot-sorted and chunk-transposed in DRAM
    ([128, nch*80] bf16) so the device streams them with plain wide
    contiguous DMAs at full line rate (no indirect DMA, no SWDGE)
  * 128-point chunks are slot-sorted, so each chunk touches a <=32-wide
    slot window (rare wider spans just emit one extra matmul); bake the
    per-matmul window-relative slot ids into a small meta tensor

Device (per core, Bass/Tile):
  * the whole [80ch x 2044slot] accumulator grid lives in 4 PSUM banks;
    one start=True zero-matmul per bank initializes it
  * per matmul: one-hot rhs [128pts, 32slots] built on DVE by is_equal
    against an iota row (batched 16 matmuls per DVE instruction); points
    tile is the stationary lhsT so the matmul moves only 32 columns
  * PSUM -> SBUF stage copies on the Activation engine per bank, then
    plain DMAs write the [80, 2044] bf16 partial back

Host combine: sum the 8 aligned partials in fp32, scatter the compact
slot rows into the zeros output grid.

A post-pass splits multi-wait instructions into single-wait
EventSemaphores (this walrus build accepts only one sync-wait slot per
instruction struct).
"""

import os
import numpy as np
import ml_dtypes

BF16 = ml_dtypes.bfloat16

# ---- problem constants (from the reference nn.Module) ----
IMAGE_SIZE = (256, 704)
FEATURE_SIZE = (32, 88)
XBOUND = (-54.0, 54.0, 0.3)
YBOUND = (-54.0, 54.0, 0.3)
ZBOUND = (-10.0, 10.0, 20.0)
DBOUND = (1.0, 60.0, 1.0)

N_CORES = 8
P = 128          # points per chunk / matmul contraction dim
OH_W = 32        # one-hot window width (moving cols per matmul)
BANK_W = 512     # PSUM bank width in fp32
KB = 16          # matmuls per batched DVE is_equal
CS = 20          # chunks per stream slab DMA
NB = 16          # slab buffers in flight
NOB = 21         # one-hot buffers in flight


def _host_geometry(img_trans, img_scale, lidar2img, B, N, D, H, W):
    """float32 numpy mirror of the reference get_geometry + voxelize."""
    dx = np.array([XBOUND[2], YBOUND[2], ZBOUND[2]], np.float32)
    bx = np.array([XBOUND[0] + XBOUND[2] / 2.0,
                   YBOUND[0] + YBOUND[2] / 2.0,
                   ZBOUND[0] + ZBOUND[2] / 2.0], np.float32)
    nx = [int((b[1] - b[0]) / b[2]) for b in (XBOUND, YBOUND, ZBOUND)]
    NX, NY, NZ = nx

    iH, iW = IMAGE_SIZE
    fH, fW = FEATURE_SIZE
    ds = np.arange(DBOUND[0], DBOUND[1], DBOUND[2], dtype=np.float32)
    xs = np.linspace(0.0, iW - 1, fW, dtype=np.float32)
    ys = np.linspace(0.0, iH - 1, fH, dtype=np.float32)
    assert ds.shape[0] == D and fH == H and fW == W

    fr = np.stack([
        np.broadcast_to(xs[None, None, :], (D, H, W)),
        np.broadcast_to(ys[None, :, None], (D, H, W)),
        np.broadcast_to(ds[:, None, None], (D, H, W)),
    ], axis=-1).astype(np.float32)                       # [D,H,W,3]

    pts = fr[None, None] + img_trans[:, :, None, None, None, :]
    d = pts[..., 2:3]
    xy = pts[..., :2] / img_scale[:, :, None, None, None, None]
    p4 = np.concatenate([xy * d, d, np.ones_like(d)], axis=-1)
    img2lidar = np.linalg.inv(lidar2img)
    geom = np.einsum('bnij,bndhwj->bndhwi', img2lidar, p4)[..., :3]
    geom = geom.astype(np.float32)
    vox = ((geom - (bx - dx / 2.0)) / dx).astype(np.int32)  # trunc toward 0
    mask = ((vox[..., 0] >= 0) & (vox[..., 0] < NX)
            & (vox[..., 1] >= 0) & (vox[..., 1] < NY)
            & (vox[..., 2] >= 0) & (vox[..., 2] < NZ))
    flat = (vox[..., 2] * NX + vox[..., 0]) * NY + vox[..., 1]
    flat = flat + np.arange(B, dtype=np.int32)[:, None, None, None, None] \
        * (NZ * NX * NY)
    flatm = np.where(mask, flat, -1).reshape(-1)
    return flatm, (NX, NY, NZ)


def _build_schedule(slot_of_pos, nch, S):
    """Per-matmul windows over the shared slot-sorted point layout.

    Returns (mm list of (chunk, bank, col_lo, width), lids [P, n_mmp] f32,
    last_mm_of_bank {bank: mm index}).
    """
    mms = []
    lid_cols = []
    for j in range(nch):
        sl = slot_of_pos[j * P:(j + 1) * P]
        real = sl[sl >= 0]
        if len(real) == 0:
            continue
        cur, hi = int(real.min()), int(real.max())
        while True:
            bank = cur // BANK_W
            wend = min(cur + OH_W, (bank + 1) * BANK_W, S)
            w = wend - cur
            lid = np.where((sl >= cur) & (sl < wend), sl - cur, -1.0)
            mms.append((j, bank, cur - bank * BANK_W, w))
            lid_cols.append(lid.astype(np.float32))
            nxt = real[real >= wend]
            if len(nxt) == 0:
                break
            cur = int(nxt.min())
    n_mm = len(mms)
    n_mmp = -(-n_mm // KB) * KB
    lids = np.full((P, n_mmp), -1.0, np.float32)
    if n_mm:
        lids[:, :n_mm] = np.stack(lid_cols, axis=1)
    last_mm_of_bank = {}
    for i, (j, bank, lo, w) in enumerate(mms):
        last_mm_of_bank[bank] = i
    return mms, lids, n_mmp, last_mm_of_bank


def _slab_plan(nch):
    """Chunk counts per slab DMA: small head (fast pipeline fill) and small
    tail (short post-stream dependency chain), CS-sized middles."""
    plan = [8, 12]
    while sum(plan) + CS + 24 <= nch:
        plan.append(CS)
    rem = nch - sum(plan)
    for part in (12, 8):
        if rem > part:
            plan.append(part)
            rem -= part
    if rem:
        plan.append(rem)
    assert sum(plan) == nch
    return plan


def _build_bass(nch, n_mmp, mms, last_mm_of_bank, S, C):
    import concourse.bass as bass
    import concourse.mybir as mybir
    import concourse.tile as tile

    f32 = mybir.dt.float32
    bf16 = mybir.dt.bfloat16
    n_banks = -(-S // BANK_W)
    # meta free layout: [iota: OH_W | lids: n_mmp]
    MW = OH_W + n_mmp
    plan = _slab_plan(nch)
    slab_of_chunk = np.repeat(np.arange(len(plan)), plan)
    slab_c0 = np.concatenate([[0], np.cumsum(plan)[:-1]])

    # per-column-range stage copy + writeback: fires as soon as the last
    # matmul touching the range retires, so only the final ~RW columns sit
    # in the post-stream tail
    RW = 256
    n_rng = -(-S // RW)
    last_mm_of_rng = {}
    for i, (j, bank, lo, w) in enumerate(mms):
        c0, c1 = bank * BANK_W + lo, bank * BANK_W + lo + w
        for r in range(c0 // RW, -(-c1 // RW)):
            last_mm_of_rng[r] = i

    nc = bass.Bass()
    pts = nc.dram_tensor("pts", [P, nch * C], bf16, kind="ExternalInput")
    meta = nc.dram_tensor("meta", [P, MW], bf16, kind="ExternalInput")
    outp = nc.dram_tensor("out", [C, S], bf16, kind="ExternalOutput")

    with tile.TileContext(nc) as tc:
        with tc.tile_pool(name="sb", bufs=1) as con, \
             tc.tile_pool(name="ps", bufs=1, space="PSUM") as ps:
            meta_sb = con.tile([P, MW], bf16, tag="meta")
            scratch = con.tile([P, BANK_W], bf16, tag="scratch")
            stage = con.tile([P, S], bf16, tag="stage")
            slabs = [con.tile([P, int(w) * C], bf16, name=f"slab{k}",
                              tag=f"slab{k}") for k, w in enumerate(plan)]
            ohs = [con.tile([P, KB * OH_W], bf16, name=f"oh{k}",
                            tag=f"oh{k}") for k in range(-(-len(mms) // KB))]
            accs = [ps.tile([P, BANK_W], f32, name=f"acc{k}", tag=f"acc{k}")
                    for k in range(n_banks)]

            # zero-matmul feed with no DMA dependency: PSUM init starts
            # during the framework preamble
            nc.gpsimd.memset(scratch[:], 0.0)
            nc.sync.dma_start(out=meta_sb[:], in_=meta[:])

            # one start=True matmul per PSUM bank zeroes the whole 2KB zero
            # region (0 x 0); afterwards arbitrary overlapping start=False
            # accumulation windows are legal
            for t in range(n_banks):
                nc.tensor.matmul(
                    out=accs[t][0:C, 0:BANK_W],
                    lhsT=scratch[:, 0:C],
                    rhs=scratch[:],
                    start=True, stop=False)

            for i, w in enumerate(plan):
                c0 = int(slab_c0[i])
                nc.sync.dma_start(out=slabs[i][:],
                                  in_=pts[:, c0 * C:(c0 + int(w)) * C])

            mstride = meta_sb[:].ap[0][0]
            for m, (j, bank, lo, w) in enumerate(mms):
                b = m // KB
                if m % KB == 0:
                    oh = ohs[b]
                    out_ap = bass.AP(oh[:].tensor, 0,
                                     [[KB * OH_W, P], [OH_W, KB], [1, OH_W]])
                    iota_ap = bass.AP(meta_sb[:].tensor, 0,
                                      [[mstride, P], [0, KB], [1, OH_W]])
                    lid_ap = bass.AP(meta_sb[:].tensor, OH_W + KB * b,
                                     [[mstride, P], [1, KB], [0, OH_W]])
                    nc.vector.tensor_tensor(
                        out=out_ap, in0=iota_ap, in1=lid_ap,
                        op=mybir.AluOpType.is_equal)
                si = int(slab_of_chunk[j])
                cj = j - int(slab_c0[si])
                nc.tensor.matmul(
                    out=accs[bank][0:C, lo:lo + w],
                    lhsT=slabs[si][:, cj * C:(cj + 1) * C],
                    rhs=ohs[b][:, (m % KB) * OH_W:(m % KB) * OH_W + w],
                    start=False, stop=(m == last_mm_of_bank[bank]))
                for r in range(n_rng):
                    if last_mm_of_rng.get(r) != m:
                        continue
                    r0 = r * RW
                    w2 = min(RW, S - r0)
                    bank_r, lo_r = r0 // BANK_W, r0 % BANK_W
                    nc.scalar.activation(
                        out=stage[0:C, r0:r0 + w2],
                        in_=accs[bank_r][0:C, lo_r:lo_r + w2],
                        func=mybir.ActivationFunctionType.Copy)
                    # issue writebacks from the Activation queue so SP stays
                    # free to pump stream slabs
                    nc.scalar.dma_start(out=outp[:, r0:r0 + w2],
                                        in_=stage[0:C, r0:r0 + w2])
    return nc


def _split_multi_waits(nc):
    """Walrus codegen allows a single sync-wait slot per instruction struct;
    hoist all but the last wait of any multi-wait instruction onto preceding
    single-wait EventSemaphore instructions on the same engine queue."""
    import concourse.mybir as mybir

    k = 0
    for bb in nc.m.functions[0].blocks:
        new = []
        changed = False
        for inst in bb.instructions:
            si = inst.sync_info
            if si is not None and si.on_wait and len(si.on_wait) > 1:
                waits = list(si.on_wait)
                for w in waits[:-1]:
                    ev = mybir.InstEventSemaphore(
                        name=f"wsplit-{k}", ins=[], outs=[])
                    k += 1
                    ev.engine = inst.engine
                    ev.sync_info = mybir.SyncInfo(on_wait=[w], on_update=[])
                    nc.inst_map[ev.name] = ev
                    new.append(ev)
                si.on_wait = [waits[-1]]
                changed = True
            new.append(inst)
        if changed:
            try:
                bb.instructions = new
            except Exception:
                bb.instructions[:] = new
    return nc


def kernel(feats, img_trans, img_scale, lidar2img):
    from concourse import bass_utils

    feats = np.ascontiguousarray(feats, dtype=np.float32)
    img_trans = np.asarray(img_trans, dtype=np.float32)
    img_scale = np.asarray(img_scale, dtype=np.float32)
    lidar2img = np.asarray(lidar2img, dtype=np.float32)
    B, N, D, H, W, C = feats.shape
    npt = B * N * D * H * W

    flatm, (NX, NY, NZ) = _host_geometry(img_trans, img_scale, lidar2img,
                                         B, N, D, H, W)
    out = np.zeros((B, NZ * C, NX, NY), np.float32)
    ib = flatm >= 0
    if not ib.any():
        return out
    uvox, slot_all = np.unique(flatm[ib], return_inverse=True)
    S = len(uvox)

    # shared padded layout: slot s owns ceil(cnt_s/8) positions on every core
    cnt = np.bincount(slot_all, minlength=S)
    m = -(-cnt // N_CORES)
    pos = np.zeros(S + 1, np.int64)
    pos[1:] = np.cumsum(m)
    M = int(pos[-1])
    nch = -(-M // P)
    Mp = nch * P
    slot_asc = np.full(Mp, -1, np.int64)
    slot_asc[:M] = np.repeat(np.arange(S), m)
    # process chunks in descending-slot order: the sparse high-slot tail
    # (many columns finishing at once) streams first and its writebacks
    # overlap the stream; the final chunks touch only the few densest
    # voxels, so the post-stream tail copies almost nothing
    slot_of_pos = slot_asc.reshape(nch, P)[::-1].reshape(-1)

    mms, lids, n_mmp, last_mm_of_bank = _build_schedule(slot_of_pos, nch, S)

    # per-voxel round-robin split of points across cores
    srt = np.argsort(slot_all, kind='stable')
    ss = slot_all[srt]
    starts = np.zeros(S, np.int64)
    starts[1:] = np.cumsum(cnt)[:-1]
    rank = np.arange(len(ss)) - starts[ss]
    core_of = rank % N_CORES
    lpos_asc = pos[ss] + rank // N_CORES
    lpos = (nch - 1 - lpos_asc // P) * P + lpos_asc % P
    feats_ib = feats.reshape(npt, C)[ib][srt].astype(BF16)

    MW = OH_W + n_mmp
    meta_np = np.zeros((P, MW), np.float32)
    meta_np[:, :OH_W] = np.arange(OH_W, dtype=np.float32)[None, :]
    meta_np[:, OH_W:] = lids
    meta_np = meta_np.astype(BF16)

    nc = _build_bass(nch, n_mmp, mms, last_mm_of_bank, S, C)
    _split_multi_waits(nc)

    in_maps = []
    for core in range(N_CORES):
        stream = np.zeros((Mp, C), BF16)
        sel = core_of == core
        stream[lpos[sel]] = feats_ib[sel]
        pts_c = np.ascontiguousarray(
            stream.reshape(nch, P, C).transpose(1, 0, 2).reshape(P, nch * C))
        in_maps.append({"pts": pts_c, "meta": meta_np})

    if bool(int(os.environ.get("BEV_TIMELINE", "0"))):
        from concourse.timeline_sim import TimelineSim
        t_ns = TimelineSim(nc).simulate()
        print(f"HW exec time: {t_ns:.0f} ns")
    res = bass_utils.run_bass_kernel_spmd(
        nc, in_maps, core_ids=list(range(N_CORES)))

    total = np.zeros((C, S), np.float64)
    for r in res.results:
        total += np.asarray(r["out"], dtype=np.float64)
    total = total.astype(np.float32)

    gsz = NZ * NX * NY
    b_u = uvox // gsz
    r_u = uvox % gsz
    z_u = r_u // (NX * NY)
    xy_u = r_u % (NX * NY)
    ov = out.reshape(B, NZ, C, NX * NY)
    ov[b_u, z_u, :, xy_u] = total.T
    return out
